# revision 2
# baseline (speedup 1.0000x reference)
"""Trainium2 Bass kernel for nn_AutodiffChannel: 6-biquad EQ cascade over
(64, 1, 262144) fp32 audio, data-parallel over 8 NeuronCores.

Algorithm (per sequence, LTI block-state decomposition):
  The 6-stage DF2T biquad cascade is a 12-state linear system
  s' = A s + B x, y = C s + D x.  Split T=262144 into 2048 chunks of
  L=128.  Then per chunk c:
      y_c = Phi x_c + Gamma S_c          (Phi  = 128x128 lower-tri Toeplitz
                                          of the impulse response h[0:128],
                                          Gamma[m,:] = C A^m)
      U_c = M x_c                        (M[:,n] = A^(127-n) B)
      S_c = sum_{j<c} (A^128)^(c-1-j) U_j   (exclusive prefix "state scan")
  The prefix is computed with a Kogge-Stone scan (11 levels) using
  precomputed powers P_d = (A^128)^(2^d).  The tiny per-sequence setup
  (h, Gamma, M, P_d) is computed host-side in float64.

Device dataflow per core (8 sequences), v2:
  x arrives as fp16 in chunk-column layout XT (column q = j*128+p holds
  chunk c = 16p+j).  All data matmuls are single-term fp16 (simulated
  end-to-end rel err ~6e-3 vs the 2e-2 gate).  Phase A computes
  U = M x for all 8 seqs into a 96-row fp32 buffer; the 11-level
  Kogge-Stone state scan runs in fp32 with fp32r-typed matmuls (1
  cyc/col on the PE for N>=256 vs 4 for fp32).  Phase B emits y
  DIRECTLY in natural layout (no PE transposes): per 128-column chunk
  block j, a matmul with the x block as the STATIONARY operand and
  Phi^T as the moving operand yields out[p, m] = y[t = 2048p+128j+m];
  the Gamma correction accumulates the same way from the q-ordered fp16
  states.  Phi matmuls are interleaved between scan levels so the PE
  works while the DVE does the scan adds.  Output is fp16 (2^-12 <<
  error budget); the host upcasts.
"""
import sys

for _p in ("/opt/trn_rl_repo", "/opt/trn_rl_repo/concourse"):
    if _p not in sys.path:
        sys.path.insert(0, _p)

import numpy as np

import concourse.bacc as bacc
import concourse.mybir as mybir
from concourse.tile import TileContext

# ---------------------------------------------------------------- problem dims
B, C, T = 64, 1, 262144
N_CORES = 8
SEQ_PER_CORE = B * C // N_CORES  # 8
L = 128                     # chunk length
NCH = T // L                # 2048 chunks per sequence
ROWS = 128                  # natural-layout partitions per sequence
COLS = T // ROWS            # 2048
JG = COLS // L              # 16 chunk-interleave factor (c = 16p + j)
LEVELS = 11                 # ceil(log2(NCH))
NSTATE = 12
F32 = mybir.dt.float32
F32R = mybir.dt.float32r
F16 = mybir.dt.float16

G16 = 16                    # chunks per local-scan group (= JG)
NG = NCH // G16             # 128 groups
TLEV = 7                    # Kogge-Stone levels over the 128 groups

PARAM_RANGES = np.array([
    [-24.0, 24.0], [20.0, 200.0], [0.1, 10.0],
    [-24.0, 24.0], [200.0, 2000.0], [0.1, 10.0],
    [-24.0, 24.0], [200.0, 2000.0], [0.1, 10.0],
    [-24.0, 24.0], [2000.0, 8000.0], [0.1, 10.0],
    [-24.0, 24.0], [4000.0, 12000.0], [0.1, 10.0],
    [-24.0, 24.0], [4000.0, 12000.0], [0.1, 10.0],
], dtype=np.float32)
FILTER_TYPES = ["low_shelf", "peaking", "peaking", "peaking", "peaking",
                "high_shelf"]


# ------------------------------------------------------------- host-side setup
def _sigmoid_f32(z):
    z = z.astype(np.float32)
    out = np.empty_like(z)
    pos = z >= 0
    out[pos] = (np.float32(1.0) / (np.float32(1.0) + np.exp(-z[pos]))).astype(
        np.float32)
    ez = np.exp(z[~pos]).astype(np.float32)
    out[~pos] = (ez / (np.float32(1.0) + ez)).astype(np.float32)
    return out


def _biquad_coeffs_f32(g, f, q, sr, ftype):
    """fp32-faithful audio-EQ-cookbook coefficients (matches reference)."""
    f32 = np.float32
    A = np.power(f32(10.0), (g / f32(40.0)).astype(f32)).astype(f32)
    w0 = (f32(2.0) * f32(np.pi) * (f / f32(sr))).astype(f32)
    alpha = (np.sin(w0, dtype=f32) / (f32(2.0) * q)).astype(f32)
    c = np.cos(w0, dtype=f32)
    sA = np.sqrt(A).astype(f32)
    one, two = f32(1.0), f32(2.0)
    if ftype == "low_shelf":
        b0 = A * ((A + one) - (A - one) * c + two * sA * alpha)
        b1 = two * A * ((A - one) - (A + one) * c)
        b2 = A * ((A + one) - (A - one) * c - two * sA * alpha)
        a0 = (A + one) + (A - one) * c + two * sA * alpha
        a1 = -two * ((A - one) + (A + one) * c)
        a2 = (A + one) + (A - one) * c - two * sA * alpha
    elif ftype == "high_shelf":
        b0 = A * ((A + one) + (A - one) * c + two * sA * alpha)
        b1 = -two * A * ((A - one) + (A + one) * c)
        b2 = A * ((A + one) + (A - one) * c - two * sA * alpha)
        a0 = (A + one) - (A - one) * c + two * sA * alpha
        a1 = two * ((A - one) - (A + one) * c)
        a2 = (A + one) - (A - one) * c - two * sA * alpha
    else:
        b0 = one + alpha * A
        b1 = -two * c
        b2 = one - alpha * A
        a0 = one + alpha / A
        a1 = -two * c
        a2 = one - alpha / A
    bc = (np.stack([b0, b1, b2], -1).astype(f32) / a0[..., None]).astype(f32)
    ac = (np.stack([a0, a1, a2], -1).astype(f32) / a0[..., None]).astype(f32)
    return bc, ac


def _coeffs_from_inputs(p, W, b, sample_rate):
    z = (p.astype(np.float32) @ W.astype(np.float32).T
         + b.astype(np.float32)).astype(np.float32)
    pn = _sigmoid_f32(z)
    lo, hi = PARAM_RANGES[:, 0], PARAM_RANGES[:, 1]
    params = (pn * (hi - lo) + lo).astype(np.float32)
    bcs, acs = [], []
    for k, ftype in enumerate(FILTER_TYPES):
        bc, ac = _biquad_coeffs_f32(
            params[:, 3 * k], params[:, 3 * k + 1], params[:, 3 * k + 2],
            float(sample_rate), ftype)
        bcs.append(bc)
        acs.append(ac)
    return np.stack(bcs), np.stack(acs)  # (6, B, 3) fp32


def _state_space(bc, ac):
    """Vectorized float64 (A, B, C, D) per sequence from fp32 DF2T coeffs."""
    nb = bc.shape[1]
    bc64 = bc.astype(np.float64)
    ac64 = ac.astype(np.float64)

    def step(s, x):
        # s: (nb, 12); x: (nb,) -> s', y
        s = s.copy()
        v = x
        for k in range(6):
            b0, b1, b2 = bc64[k, :, 0], bc64[k, :, 1], bc64[k, :, 2]
            a1, a2 = ac64[k, :, 1], ac64[k, :, 2]
            s1, s2 = s[:, 2 * k], s[:, 2 * k + 1]
            y = b0 * v + s1
            s[:, 2 * k] = b1 * v - a1 * y + s2
            s[:, 2 * k + 1] = b2 * v - a2 * y
            v = y
        return s, v

    A = np.zeros((nb, NSTATE, NSTATE))
    Cv = np.zeros((nb, NSTATE))
    for i in range(NSTATE):
        e = np.zeros((nb, NSTATE))
        e[:, i] = 1.0
        sp, y = step(e, np.zeros(nb))
        A[:, :, i] = sp
        Cv[:, i] = y
    Bv, D = step(np.zeros((nb, NSTATE)), np.ones(nb))
    return A, Bv, Cv, D


def _dlyap(A, Q):
    """Discrete Lyapunov: X - A X A^T = Q, per batch, via kron solve."""
    nb, n, _ = A.shape
    I = np.eye(n * n)
    K = I - np.einsum("bij,bkl->bikjl", A, A).reshape(nb, n * n, n * n)
    X = np.linalg.solve(K, Q.reshape(nb, n * n, 1)).reshape(nb, n, n)
    return 0.5 * (X + X.transpose(0, 2, 1))


def _balance(A, Bv, Cv):
    """Per-seq balanced realization (square-root algorithm).  The balanced
    transition matrix is a contraction, so all its powers have entries <= 1
    and fp16-rounded powers are numerically safe in the scan."""
    nb = A.shape[0]
    Wc = _dlyap(A, np.einsum("bi,bj->bij", Bv, Bv))
    Wo = _dlyap(A.transpose(0, 2, 1), np.einsum("bi,bj->bij", Cv, Cv))
    Ab = np.zeros_like(A)
    Bb = np.zeros_like(Bv)
    Cb = np.zeros_like(Cv)
    for g in range(nb):
        jit = 1e-14 * max(np.trace(Wc[g]), np.trace(Wo[g]), 1e-30)
        for _ in range(8):
            try:
                Lc = np.linalg.cholesky(Wc[g] + jit * np.eye(NSTATE))
                Lo = np.linalg.cholesky(Wo[g] + jit * np.eye(NSTATE))
                break
            except np.linalg.LinAlgError:
                jit *= 100.0
        U2, s, V2t = np.linalg.svd(Lo.T @ Lc)
        s = np.maximum(s, 1e-12)
        Rinv = (s[:, None] ** -0.5) * (U2.T @ Lo.T)
        R = (Lc @ V2t.T) * (s[None, :] ** -0.5)
        Ab[g] = Rinv @ A[g] @ R
        Bb[g] = Rinv @ Bv[g]
        Cb[g] = Cv[g] @ R
    return Ab, Bb, Cb


def _derived(A, Bv, Cv, D):
    """h (nb,L), Gamma (nb,L,12), M (nb,12,L), Pd (nb,LEVELS,12,12) in f64."""
    nb = A.shape[0]
    h = np.zeros((nb, L))
    Gam = np.zeros((nb, L, NSTATE))
    M = np.zeros((nb, NSTATE, L))
    h[:, 0] = D
    cam = Cv.copy()          # C A^m
    amb = Bv.copy()          # A^m B
    for m in range(L):
        Gam[:, m, :] = cam
        M[:, :, L - 1 - m] = amb
        if m + 1 < L:
            h[:, m + 1] = np.einsum("bi,bi->b", cam, Bv)
        cam = np.einsum("bi,bij->bj", cam, A)
        amb = np.einsum("bij,bj->bi", A, amb)
    sq = A.copy()
    for _ in range(7):       # A^(2^7) = A^128
        sq = sq @ sq
    Pd = np.zeros((nb, LEVELS, NSTATE, NSTATE))
    for d in range(LEVELS):
        Pd[:, d] = sq
        sq = sq @ sq
    return h, Gam, M, Pd


def _pack_weights(h, Gam, M, Pd, Ek):
    """Device weight tensors, per core.  Ek: (nb, 16, 12, 12) = E^k powers."""
    nb = h.shape[0]
    m_idx = np.arange(L)
    diff = m_idx[None, :] - m_idx[:, None]          # [n, m] = m - n
    phiT = np.where(diff >= 0, h[:, np.clip(diff, 0, L - 1)],
                    0.0).astype(np.float16)         # (nb, n=128, m=128)
    # embedded at per-seq 12-row offsets inside a 96-row frame so every
    # device access stays at base partition 0 (HW requires 32-aligned bases)
    g2 = np.zeros((nb, 96, L), np.float16)          # (nb, k-embed, m)
    mT = np.zeros((nb, L, 96), np.float16)          # (nb, n, k-embed)
    for g in range(nb):
        s8 = g % SEQ_PER_CORE
        g2[g, 12 * s8:12 * s8 + 12, :] = Gam[g].T.astype(np.float16)
        mT[g, :, 12 * s8:12 * s8 + 12] = M[g].T.astype(np.float16)
    # block-diagonal (E^k)^T for the local scan / assemble, fp16
    ekT = np.zeros((N_CORES, G16, 96, 96), np.float16)
    # block-diagonal ((E^16)^(2^d))^T for the group scan, fp32 (= Pd[4+d])
    tscanP = np.zeros((N_CORES, TLEV, 96, 96), np.float32)
    for core in range(N_CORES):
        for s in range(SEQ_PER_CORE):
            g = core * SEQ_PER_CORE + s
            sl = slice(12 * s, 12 * s + 12)
            for k in range(G16):
                ekT[core, k, sl, sl] = Ek[g, k].T.astype(np.float16)
            for d in range(TLEV):
                tscanP[core, d, sl, sl] = Pd[g, 4 + d].T.astype(np.float32)
    return phiT, g2, mT, ekT, tscanP


# ------------------------------------------------------------ device kernel IR
_NC_CACHE = {}


def build_nc(rep=1):
    key = rep
    if key in _NC_CACHE:
        return _NC_CACHE[key]
    nc = bacc.Bacc("TRN2")
    xt_d = nc.dram_tensor("xt", [SEQ_PER_CORE, ROWS, COLS], F16,
                          kind="ExternalInput")
    phiT_d = nc.dram_tensor("phiT", [SEQ_PER_CORE, L, L], F16,
                            kind="ExternalInput")
    g2_d = nc.dram_tensor("g2", [SEQ_PER_CORE, 96, L], F16,
                          kind="ExternalInput")
    mT_d = nc.dram_tensor("mT", [SEQ_PER_CORE, L, 96], F16,
                          kind="ExternalInput")
    ekT_d = nc.dram_tensor("ekT", [G16, 96, 96], F16, kind="ExternalInput")
    tscanP_d = nc.dram_tensor("tscanP", [TLEV, 96, 96], F32,
                              kind="ExternalInput")
    y_d = nc.dram_tensor("y", [SEQ_PER_CORE, ROWS, COLS], F16,
                         kind="ExternalOutput")

    with TileContext(nc) as tc:
        with tc.tile_pool(name="weights", bufs=1) as wpool:
            # mT first: phase A (U = M x) is the first consumer; the other
            # weights are needed later and must not delay the xt DMAs
            mT_sb = wpool.tile([L, SEQ_PER_CORE * 96], F16)
            nc.sync.dma_start(
                out=mT_sb[:].rearrange("n (s k) -> n s k", k=96),
                in_=mT_d[:].transpose([1, 0, 2]))
            phiT_sb = wpool.tile([L, SEQ_PER_CORE * L], F16)
            g2_sb = wpool.tile([96, SEQ_PER_CORE * L], F16)
            ekT_sb = wpool.tile([96, G16 * 96], F16)
            tscanP_sb = wpool.tile([96, TLEV * 96], F32)

            def late_weight_dmas():
                nc.sync.dma_start(
                    out=ekT_sb[:].rearrange("j (d k) -> j d k", k=96),
                    in_=ekT_d[:].transpose([1, 0, 2]))
                nc.sync.dma_start(
                    out=phiT_sb[:].rearrange("p (s m) -> p s m", m=L),
                    in_=phiT_d[:].transpose([1, 0, 2]))
                nc.sync.dma_start(
                    out=tscanP_sb[:].rearrange("j (d k) -> j d k", k=96),
                    in_=tscanP_d[:].transpose([1, 0, 2]))
                nc.sync.dma_start(
                    out=g2_sb[:].rearrange("k (s m) -> k s m", m=L),
                    in_=g2_d[:].transpose([1, 0, 2]))

            # bufs=2 on xt/ysb: rep k+1's input DMAs and phase A overlap
            # rep k's phase-B tail (steady-state pipelining across reps)
            with tc.tile_pool(name="xt", bufs=2) as xtpool, \
                 tc.tile_pool(name="ysb", bufs=2) as ypool, \
                 tc.tile_pool(name="state", bufs=1) as stpool:
                for _ in range(rep):
                    _one_pass(nc, tc, xt_d, y_d, phiT_sb, g2_sb, mT_sb,
                              ekT_sb, tscanP_sb, xtpool, ypool, stpool,
                              late_weight_dmas)
                    late_weight_dmas = None
    nc.compile()
    _NC_CACHE[key] = nc
    return nc


def _one_pass(nc, tc, xt_d, y_d, phiT_sb, g2_sb, mT_sb, ekT_sb, tscanP_sb,
              xtpool, ypool, stpool, late_weight_dmas=None):
    XT = [xtpool.tile([ROWS, COLS], F16, tag=f"xt{s}", name=f"xt{s}")
          for s in range(SEQ_PER_CORE)]
    YS = [ypool.tile([ROWS, COLS], F16, tag=f"ys{s}", name=f"ys{s}")
          for s in range(SEQ_PER_CORE)]
    # Uq: q-ordered fp16 U, col j*128+p <-> chunk c = 16p+j;
    # rows 12s..12s+12 = seq s.
    Uq = stpool.tile([96, NCH], F16, tag="uq")
    Sq = stpool.tile([96, NCH], F16, tag="sq")
    Bb = stpool.tile([96, NG], F32, tag="bb")     # group-scan state (f32)
    B16 = stpool.tile([96, NG], F16, tag="b16")   # exclusive, fp16

    for sq in range(SEQ_PER_CORE):
        nc.sync.dma_start(out=XT[sq], in_=xt_d[sq])
    if late_weight_dmas is not None:
        late_weight_dmas()

    # ---- phase A: U = M x, all 8 seqs accumulated into 96-row psum tiles;
    # psum block i holds q-columns [512i, 512i+512) contiguously.
    with tc.tile_pool(name="up", bufs=1, space="PSUM") as upsum:
        ups = [upsum.tile([96, 512], F32, tag=f"up{i}", name=f"up{i}")
               for i in range(4)]
        for sq in range(SEQ_PER_CORE):
            for i in range(4):
                nc.tensor.matmul(
                    ups[i][:],
                    lhsT=mT_sb[:, sq * 96:(sq + 1) * 96],
                    rhs=XT[sq][:, i * 512:(i + 1) * 512],
                    start=(sq == 0), stop=(sq == SEQ_PER_CORE - 1))
        for i in range(4):
            eng = nc.vector if i % 2 == 0 else nc.scalar
            if eng is nc.vector:
                eng.tensor_copy(out=Uq[:, 512 * i:512 * (i + 1)],
                                in_=ups[i][:])
            else:
                eng.copy(Uq[:, 512 * i:512 * (i + 1)], ups[i][:])

    with tc.tile_pool(name="sqp", bufs=1, space="PSUM") as sqpool, \
         tc.tile_pool(name="vt", bufs=1, space="PSUM") as vtpool, \
         tc.tile_pool(name="yp", bufs=3, space="PSUM") as yppool:
        # ---- group sums: V_p = sum_j E^(15-j) U_(16p+j), PSUM-accumulated
        V = vtpool.tile([96, NG], F32, tag="vt", name="vt")
        for jp in range(G16):
            k = 15 - jp
            nc.tensor.matmul(
                V[:],
                lhsT=ekT_sb[:, k * 96:(k + 1) * 96],
                rhs=Uq[:, 128 * jp:128 * (jp + 1)],
                start=(jp == 0), stop=(jp == G16 - 1))
        nc.vector.tensor_copy(out=Bb[:, :], in_=V[:])

        # ---- assemble U-terms into SQ psum: block j += E^(j-1-j') Uq_(j'),
        # emitted k-major (k = j-1-j' >= 1; the k = 0 term is folded into
        # the final DVE add as a plain Uq block shift).  These only need Uq,
        # so they fill the PE while the group scan below runs.
        sqt = sqpool.tile([96, NCH], F32, tag="sqp", name="sqt")
        # for fixed k, the out blocks j = k+1..15 and rhs blocks j-1-k =
        # 0..14-k are both contiguous col spans: one shifted matmul per k,
        # split only at 512-col psum bank boundaries
        uterms = []
        for k in range(1, 15):
            o0, o1 = 128 * (k + 1), NCH
            c0 = o0
            while c0 < o1:
                c1 = min((c0 // 512 + 1) * 512, o1)
                uterms.append((k, c0, c1))
                c0 = c1
        ut_idx = [0]

        def emit_uterms(n):
            for _ in range(n):
                if ut_idx[0] >= len(uterms):
                    return
                k, c0, c1 = uterms[ut_idx[0]]
                ut_idx[0] += 1
                sh = 128 * (k + 1)
                nc.tensor.matmul(
                    sqt[:, c0:c1],
                    lhsT=ekT_sb[:, k * 96:(k + 1) * 96],
                    rhs=Uq[:, c0 - sh:c1 - sh],
                    start=(k == 1), stop=False)

        # ---- group scan: 7-level KS in f32, transitions ((E^16)^(2^d))^T,
        # interleaved with the U-term matmuls (PE fill during DVE adds)
        for d in range(TLEV):
            sh = 1 << d
            tsp = vtpool.tile([96, NG], F32, tag="vt", name="tsp")
            nc.tensor.matmul(
                tsp[:, 0:NG - sh],
                lhsT=tscanP_sb[:, d * 96:(d + 1) * 96],
                rhs=Bb[:, 0:NG - sh],
                start=True, stop=True)
            emit_uterms(3)
            nc.vector.tensor_add(out=Bb[:, sh:NG], in0=Bb[:, sh:NG],
                                 in1=tsp[:, 0:NG - sh])
        emit_uterms(len(uterms))  # leftovers
        # exclusive shift + fp16: B16 = [0, Bb[0..NG-2]]
        nc.vector.memset(B16[:, 0:1], 0.0)
        nc.vector.tensor_copy(out=B16[:, 1:NG], in_=Bb[:, 0:NG - 1])

        # ---- T-terms: block j += E^j * B16.  PSUM group discipline: each
        # 2KB bank (4 blocks) has ONE group, started by its k==1 U-term
        # (which marks the whole bank pending-zero, so blocks 0/1 overwrite
        # zeros correctly) and stopped by the bank's last T-term.
        for j in range(G16):
            nc.tensor.matmul(
                sqt[:, 128 * j:128 * (j + 1)],
                lhsT=ekT_sb[:, j * 96:(j + 1) * 96],
                rhs=B16[:, :],
                start=False, stop=(j % 4 == 3))

        # ---- final states: Sq block j = psum block j + Uq block j-1
        nc.scalar.copy(Sq[:, 0:128], sqt[:, 0:128])
        nc.vector.tensor_add(out=Sq[:, 128:NCH], in0=Uq[:, 0:NCH - 128],
                             in1=sqt[:, 128:NCH])

        # ---- phase B: y = Phi x + Gamma S fused per 512-col psum tile,
        # then a single psum -> fp16 copy (ACT/DVE alternating)
        for s in range(SEQ_PER_CORE):
            for jg in range(4):
                gp = yppool.tile([128, 512], F32, tag="yp", name="gp")
                # one psum group per 512-col bank: first phi mm starts (bank
                # marked pending-zero), last gamma mm stops
                for jj in range(4):
                    j = 4 * jg + jj
                    nc.tensor.matmul(
                        gp[:, 128 * jj:128 * (jj + 1)],
                        lhsT=XT[s][:, 128 * j:128 * (j + 1)],
                        rhs=phiT_sb[:, 128 * s:128 * (s + 1)],
                        start=(jj == 0), stop=False)
                for jj in range(4):
                    j = 4 * jg + jj
                    nc.tensor.matmul(
                        gp[:, 128 * jj:128 * (jj + 1)],
                        lhsT=Sq[:, 128 * j:128 * (j + 1)],
                        rhs=g2_sb[:, 128 * s:128 * (s + 1)],
                        start=False, stop=(jj == 3))
                if (4 * s + jg) % 2 == 0:
                    nc.vector.tensor_copy(
                        out=YS[s][:, 512 * jg:512 * (jg + 1)], in_=gp[:])
                else:
                    nc.scalar.copy(YS[s][:, 512 * jg:512 * (jg + 1)], gp[:])
            nc.sync.dma_start(out=y_d[s], in_=YS[s])


# ----------------------------------------------------------------- entry point
class BassRunner:
    """Builds the sharded jitted executable for a compiled Bass module once;
    subsequent calls only device_put inputs and execute."""

    def __init__(self, nc, n_cores=N_CORES):
        import jax
        from jax.experimental.shard_map import shard_map
        from jax.sharding import Mesh, PartitionSpec
        from concourse.bass2jax import (_bass_exec_p, install_neuronx_cc_hook,
                                        partition_id_tensor)
        install_neuronx_cc_hook()
        self.jax = jax
        partition_name = (nc.partition_id_tensor.name
                          if nc.partition_id_tensor else None)
        in_names, out_names, out_avals, zero_outs = [], [], [], []
        for alloc in nc.m.functions[0].allocations:
            if not isinstance(alloc, mybir.MemoryLocationSet):
                continue
            name = alloc.memorylocations[0].name
            if alloc.kind == "ExternalInput":
                if name != partition_name:
                    in_names.append(name)
            elif alloc.kind == "ExternalOutput":
                out_names.append(name)
                shape = tuple(alloc.tensor_shape)
                dtype = mybir.dt.np(alloc.dtype)
                out_avals.append(jax.core.ShapedArray(shape, dtype))
                zero_outs.append(np.zeros(shape, dtype))
        self.in_names, self.out_names = in_names, out_names
        self.out_avals, self.zero_outs = out_avals, zero_outs
        all_in_names = list(in_names) + list(out_names)
        if partition_name is not None:
            all_in_names.append(partition_name)

        def _body(*args):
            operands = list(args)
            if partition_name is not None:
                operands.append(partition_id_tensor())
            return tuple(_bass_exec_p.bind(
                *operands, out_avals=tuple(out_avals),
                in_names=tuple(all_in_names), out_names=tuple(out_names),
                lowering_input_output_aliases=(),
                sim_require_finite=True, sim_require_nnan=True, nc=nc))

        devices = jax.devices()[:n_cores]
        mesh = Mesh(np.asarray(devices), ("core",))
        nin = len(in_names) + len(out_names)
        self.fn = jax.jit(
            shard_map(_body, mesh=mesh,
                      in_specs=(PartitionSpec("core"),) * nin,
                      out_specs=(PartitionSpec("core"),) * len(out_names),
                      check_rep=False),
            keep_unused=True)
        self.n_cores = n_cores

    def concat_args(self, in_maps):
        args = [np.concatenate([np.asarray(in_maps[c][nm])
                                for c in range(self.n_cores)], axis=0)
                for nm in self.in_names]
        args += [np.zeros((self.n_cores * z.shape[0], *z.shape[1:]), z.dtype)
                 for z in self.zero_outs]
        return args

    def __call__(self, in_maps):
        outs = self.fn(*self.concat_args(in_maps))
        self.jax.block_until_ready(outs)
        return outs


_RUNNER_CACHE = {}


def _get_runner(rep=1):
    if rep not in _RUNNER_CACHE:
        _RUNNER_CACHE[rep] = BassRunner(build_nc(rep=rep))
    return _RUNNER_CACHE[rep]


def _prepare_in_maps(x, p, W, b, sample_rate):
    bc, ac = _coeffs_from_inputs(p, W, b, sample_rate)
    A, Bv, Cv, D = _state_space(bc, ac)
    Ab, Bb, Cb = _balance(A, Bv, Cv)
    h, Gam, M, Pd = _derived(Ab, Bb, Cb, D)
    nb = Ab.shape[0]
    E = Pd[:, 0]                       # A^128 (balanced basis)
    Ek = np.zeros((nb, G16, NSTATE, NSTATE))
    Ek[:, 0] = np.eye(NSTATE)
    for k in range(1, G16):
        Ek[:, k] = np.einsum("bij,bjk->bik", E, Ek[:, k - 1])
    phiT, g2, mT, ekT, tscanP = _pack_weights(h, Gam, M, Pd, Ek)
    # chunk-column layout: xt[s][n, j*128+p] = x[s, 2048p + 128j + n]
    # (chunk c = 16p + j at column q = j*128 + p, matching the device views)
    x4 = x.reshape(B * C, ROWS, JG, L).astype(np.float32)
    xt = np.ascontiguousarray(x4.transpose(0, 3, 2, 1)).reshape(
        B * C, L, COLS).astype(np.float16)
    in_maps = []
    for core in range(N_CORES):
        sl = slice(core * SEQ_PER_CORE, (core + 1) * SEQ_PER_CORE)
        in_maps.append({
            "xt": np.ascontiguousarray(xt[sl]),
            "phiT": np.ascontiguousarray(phiT[sl]),
            "g2": np.ascontiguousarray(g2[sl]),
            "mT": np.ascontiguousarray(mT[sl]),
            "ekT": np.ascontiguousarray(ekT[core]),
            "tscanP": np.ascontiguousarray(tscanP[core]),
        })
    return in_maps


def kernel(x, p, W, b, sample_rate):
    runner = _get_runner(rep=1)
    in_maps = _prepare_in_maps(x, p, W, b, sample_rate)
    outs = runner(in_maps)
    y = np.asarray(outs[0]).astype(np.float32).reshape(B * C, T)
    return y.reshape(B, C, T)


# revision 4
# speedup vs baseline: 1.0964x; 1.0964x over previous
"""Trainium2 Bass kernel for nn_AutodiffChannel: 6-biquad EQ cascade over
(64, 1, 262144) fp32 audio, data-parallel over 8 NeuronCores.

Algorithm (per sequence, LTI block-state decomposition):
  The 6-stage DF2T biquad cascade is a 12-state linear system
  s' = A s + B x, y = C s + D x.  Split T=262144 into 2048 chunks of
  L=128.  Then per chunk c:
      y_c = Phi x_c + Gamma S_c          (Phi  = 128x128 lower-tri Toeplitz
                                          of the impulse response h[0:128],
                                          Gamma[m,:] = C A^m)
      U_c = M x_c                        (M[:,n] = A^(127-n) B)
      S_c = sum_{j<c} (A^128)^(c-1-j) U_j   (exclusive prefix "state scan")
  The prefix is computed with a Kogge-Stone scan (11 levels) using
  precomputed powers P_d = (A^128)^(2^d).  The tiny per-sequence setup
  (h, Gamma, M, P_d) is computed host-side in float64.

Device dataflow per core (8 sequences), v2:
  x arrives as fp16 in chunk-column layout XT (column q = j*128+p holds
  chunk c = 16p+j).  All data matmuls are single-term fp16 (simulated
  end-to-end rel err ~6e-3 vs the 2e-2 gate).  Phase A computes
  U = M x for all 8 seqs into a 96-row fp32 buffer; the 11-level
  Kogge-Stone state scan runs in fp32 with fp32r-typed matmuls (1
  cyc/col on the PE for N>=256 vs 4 for fp32).  Phase B emits y
  DIRECTLY in natural layout (no PE transposes): per 128-column chunk
  block j, a matmul with the x block as the STATIONARY operand and
  Phi^T as the moving operand yields out[p, m] = y[t = 2048p+128j+m];
  the Gamma correction accumulates the same way from the q-ordered fp16
  states.  Phi matmuls are interleaved between scan levels so the PE
  works while the DVE does the scan adds.  Output is fp16 (2^-12 <<
  error budget); the host upcasts.
"""
import sys

for _p in ("/opt/trn_rl_repo", "/opt/trn_rl_repo/concourse"):
    if _p not in sys.path:
        sys.path.insert(0, _p)

import numpy as np

import concourse.bacc as bacc
import concourse.mybir as mybir
from concourse.tile import TileContext

# ---------------------------------------------------------------- problem dims
B, C, T = 64, 1, 262144
N_CORES = 8
SEQ_PER_CORE = B * C // N_CORES  # 8
L = 128                     # chunk length
NCH = T // L                # 2048 chunks per sequence
ROWS = 128                  # natural-layout partitions per sequence
COLS = T // ROWS            # 2048
JG = COLS // L              # 16 chunk-interleave factor (c = 16p + j)
LEVELS = 11                 # ceil(log2(NCH))
NSTATE = 12
F32 = mybir.dt.float32
F32R = mybir.dt.float32r
F16 = mybir.dt.float16

G16 = 16                    # chunks per local-scan group (= JG)
NG = NCH // G16             # 128 groups
TLEV = 7                    # Kogge-Stone levels over the 128 groups

PARAM_RANGES = np.array([
    [-24.0, 24.0], [20.0, 200.0], [0.1, 10.0],
    [-24.0, 24.0], [200.0, 2000.0], [0.1, 10.0],
    [-24.0, 24.0], [200.0, 2000.0], [0.1, 10.0],
    [-24.0, 24.0], [2000.0, 8000.0], [0.1, 10.0],
    [-24.0, 24.0], [4000.0, 12000.0], [0.1, 10.0],
    [-24.0, 24.0], [4000.0, 12000.0], [0.1, 10.0],
], dtype=np.float32)
FILTER_TYPES = ["low_shelf", "peaking", "peaking", "peaking", "peaking",
                "high_shelf"]


# ------------------------------------------------------------- host-side setup
def _sigmoid_f32(z):
    z = z.astype(np.float32)
    out = np.empty_like(z)
    pos = z >= 0
    out[pos] = (np.float32(1.0) / (np.float32(1.0) + np.exp(-z[pos]))).astype(
        np.float32)
    ez = np.exp(z[~pos]).astype(np.float32)
    out[~pos] = (ez / (np.float32(1.0) + ez)).astype(np.float32)
    return out


def _biquad_coeffs_f32(g, f, q, sr, ftype):
    """fp32-faithful audio-EQ-cookbook coefficients (matches reference)."""
    f32 = np.float32
    A = np.power(f32(10.0), (g / f32(40.0)).astype(f32)).astype(f32)
    w0 = (f32(2.0) * f32(np.pi) * (f / f32(sr))).astype(f32)
    alpha = (np.sin(w0, dtype=f32) / (f32(2.0) * q)).astype(f32)
    c = np.cos(w0, dtype=f32)
    sA = np.sqrt(A).astype(f32)
    one, two = f32(1.0), f32(2.0)
    if ftype == "low_shelf":
        b0 = A * ((A + one) - (A - one) * c + two * sA * alpha)
        b1 = two * A * ((A - one) - (A + one) * c)
        b2 = A * ((A + one) - (A - one) * c - two * sA * alpha)
        a0 = (A + one) + (A - one) * c + two * sA * alpha
        a1 = -two * ((A - one) + (A + one) * c)
        a2 = (A + one) + (A - one) * c - two * sA * alpha
    elif ftype == "high_shelf":
        b0 = A * ((A + one) + (A - one) * c + two * sA * alpha)
        b1 = -two * A * ((A - one) + (A + one) * c)
        b2 = A * ((A + one) + (A - one) * c - two * sA * alpha)
        a0 = (A + one) - (A - one) * c + two * sA * alpha
        a1 = two * ((A - one) - (A + one) * c)
        a2 = (A + one) - (A - one) * c - two * sA * alpha
    else:
        b0 = one + alpha * A
        b1 = -two * c
        b2 = one - alpha * A
        a0 = one + alpha / A
        a1 = -two * c
        a2 = one - alpha / A
    bc = (np.stack([b0, b1, b2], -1).astype(f32) / a0[..., None]).astype(f32)
    ac = (np.stack([a0, a1, a2], -1).astype(f32) / a0[..., None]).astype(f32)
    return bc, ac


def _coeffs_from_inputs(p, W, b, sample_rate):
    z = (p.astype(np.float32) @ W.astype(np.float32).T
         + b.astype(np.float32)).astype(np.float32)
    pn = _sigmoid_f32(z)
    lo, hi = PARAM_RANGES[:, 0], PARAM_RANGES[:, 1]
    params = (pn * (hi - lo) + lo).astype(np.float32)
    bcs, acs = [], []
    for k, ftype in enumerate(FILTER_TYPES):
        bc, ac = _biquad_coeffs_f32(
            params[:, 3 * k], params[:, 3 * k + 1], params[:, 3 * k + 2],
            float(sample_rate), ftype)
        bcs.append(bc)
        acs.append(ac)
    return np.stack(bcs), np.stack(acs)  # (6, B, 3) fp32


def _state_space(bc, ac):
    """Vectorized float64 (A, B, C, D) per sequence from fp32 DF2T coeffs."""
    nb = bc.shape[1]
    bc64 = bc.astype(np.float64)
    ac64 = ac.astype(np.float64)

    def step(s, x):
        # s: (nb, 12); x: (nb,) -> s', y
        s = s.copy()
        v = x
        for k in range(6):
            b0, b1, b2 = bc64[k, :, 0], bc64[k, :, 1], bc64[k, :, 2]
            a1, a2 = ac64[k, :, 1], ac64[k, :, 2]
            s1, s2 = s[:, 2 * k], s[:, 2 * k + 1]
            y = b0 * v + s1
            s[:, 2 * k] = b1 * v - a1 * y + s2
            s[:, 2 * k + 1] = b2 * v - a2 * y
            v = y
        return s, v

    A = np.zeros((nb, NSTATE, NSTATE))
    Cv = np.zeros((nb, NSTATE))
    for i in range(NSTATE):
        e = np.zeros((nb, NSTATE))
        e[:, i] = 1.0
        sp, y = step(e, np.zeros(nb))
        A[:, :, i] = sp
        Cv[:, i] = y
    Bv, D = step(np.zeros((nb, NSTATE)), np.ones(nb))
    return A, Bv, Cv, D


def _dlyap(A, Q):
    """Discrete Lyapunov: X - A X A^T = Q, per batch, via kron solve."""
    nb, n, _ = A.shape
    I = np.eye(n * n)
    K = I - np.einsum("bij,bkl->bikjl", A, A).reshape(nb, n * n, n * n)
    X = np.linalg.solve(K, Q.reshape(nb, n * n, 1)).reshape(nb, n, n)
    return 0.5 * (X + X.transpose(0, 2, 1))


def _balance(A, Bv, Cv):
    """Per-seq balanced realization (square-root algorithm).  The balanced
    transition matrix is a contraction, so all its powers have entries <= 1
    and fp16-rounded powers are numerically safe in the scan."""
    nb = A.shape[0]
    Wc = _dlyap(A, np.einsum("bi,bj->bij", Bv, Bv))
    Wo = _dlyap(A.transpose(0, 2, 1), np.einsum("bi,bj->bij", Cv, Cv))
    Ab = np.zeros_like(A)
    Bb = np.zeros_like(Bv)
    Cb = np.zeros_like(Cv)
    for g in range(nb):
        jit = 1e-14 * max(np.trace(Wc[g]), np.trace(Wo[g]), 1e-30)
        for _ in range(8):
            try:
                Lc = np.linalg.cholesky(Wc[g] + jit * np.eye(NSTATE))
                Lo = np.linalg.cholesky(Wo[g] + jit * np.eye(NSTATE))
                break
            except np.linalg.LinAlgError:
                jit *= 100.0
        U2, s, V2t = np.linalg.svd(Lo.T @ Lc)
        s = np.maximum(s, 1e-12)
        Rinv = (s[:, None] ** -0.5) * (U2.T @ Lo.T)
        R = (Lc @ V2t.T) * (s[None, :] ** -0.5)
        Ab[g] = Rinv @ A[g] @ R
        Bb[g] = Rinv @ Bv[g]
        Cb[g] = Cv[g] @ R
    return Ab, Bb, Cb


def _derived(A, Bv, Cv, D):
    """h (nb,L), Gamma (nb,L,12), M (nb,12,L), Pd (nb,LEVELS,12,12) in f64."""
    nb = A.shape[0]
    h = np.zeros((nb, L))
    Gam = np.zeros((nb, L, NSTATE))
    M = np.zeros((nb, NSTATE, L))
    h[:, 0] = D
    cam = Cv.copy()          # C A^m
    amb = Bv.copy()          # A^m B
    for m in range(L):
        Gam[:, m, :] = cam
        M[:, :, L - 1 - m] = amb
        if m + 1 < L:
            h[:, m + 1] = np.einsum("bi,bi->b", cam, Bv)
        cam = np.einsum("bi,bij->bj", cam, A)
        amb = np.einsum("bij,bj->bi", A, amb)
    sq = A.copy()
    for _ in range(7):       # A^(2^7) = A^128
        sq = sq @ sq
    Pd = np.zeros((nb, LEVELS, NSTATE, NSTATE))
    for d in range(LEVELS):
        Pd[:, d] = sq
        sq = sq @ sq
    return h, Gam, M, Pd


def _pack_weights(h, Gam, M, Pd, Ek):
    """Device weight tensors, per core.  Ek: (nb, 16, 12, 12) = E^k powers."""
    nb = h.shape[0]
    m_idx = np.arange(L)
    diff = m_idx[None, :] - m_idx[:, None]          # [n, m] = m - n
    phiT = np.where(diff >= 0, h[:, np.clip(diff, 0, L - 1)],
                    0.0).astype(np.float16)         # (nb, n=128, m=128)
    # embedded at per-seq 12-row offsets inside a 96-row frame so every
    # device access stays at base partition 0 (HW requires 32-aligned bases)
    g2 = np.zeros((nb, 96, L), np.float16)          # (nb, k-embed, m)
    mT = np.zeros((nb, L, 96), np.float16)          # (nb, n, k-embed)
    for g in range(nb):
        s8 = g % SEQ_PER_CORE
        g2[g, 12 * s8:12 * s8 + 12, :] = Gam[g].T.astype(np.float16)
        mT[g, :, 12 * s8:12 * s8 + 12] = M[g].T.astype(np.float16)
    # block-diagonal (E^k)^T for the local scan / assemble, fp16
    ekT = np.zeros((N_CORES, G16, 96, 128), np.float16)
    # block-diagonal ((E^16)^(2^d))^T for the group scan, fp32 (= Pd[4+d])
    tscanP = np.zeros((N_CORES, TLEV, 96, 96), np.float32)
    for core in range(N_CORES):
        for s in range(SEQ_PER_CORE):
            g = core * SEQ_PER_CORE + s
            sl = slice(12 * s, 12 * s + 12)
            for k in range(G16):
                ekT[core, k, sl, sl] = Ek[g, k].T.astype(np.float16)
            for d in range(TLEV):
                tscanP[core, d, sl, sl] = Pd[g, 4 + d].T.astype(np.float32)
    return phiT, g2, mT, ekT, tscanP


# ------------------------------------------------------------ device kernel IR
_NC_CACHE = {}


def build_nc(rep=1):
    key = rep
    if key in _NC_CACHE:
        return _NC_CACHE[key]
    nc = bacc.Bacc("TRN2")
    xt_d = nc.dram_tensor("xt", [SEQ_PER_CORE, ROWS, COLS], F16,
                          kind="ExternalInput")
    phiT_d = nc.dram_tensor("phiT", [SEQ_PER_CORE, L, L], F16,
                            kind="ExternalInput")
    g2_d = nc.dram_tensor("g2", [SEQ_PER_CORE, 96, L], F16,
                          kind="ExternalInput")
    mT_d = nc.dram_tensor("mT", [SEQ_PER_CORE, L, 96], F16,
                          kind="ExternalInput")
    ekT_d = nc.dram_tensor("ekT", [G16, 96, 128], F16, kind="ExternalInput")
    tscanP_d = nc.dram_tensor("tscanP", [TLEV, 96, 96], F32,
                              kind="ExternalInput")
    y_d = nc.dram_tensor("y", [SEQ_PER_CORE, ROWS, COLS], F16,
                         kind="ExternalOutput")

    with TileContext(nc) as tc:
        with tc.tile_pool(name="weights", bufs=1) as wpool:
            # mT first: phase A (U = M x) is the first consumer; the other
            # weights are needed later and must not delay the xt DMAs
            mT_sb = wpool.tile([L, SEQ_PER_CORE * 96], F16)
            nc.sync.dma_start(
                out=mT_sb[:].rearrange("n (s k) -> n s k", k=96),
                in_=mT_d[:].transpose([1, 0, 2]))
            phiT_sb = wpool.tile([L, SEQ_PER_CORE * L], F16)
            g2_sb = wpool.tile([96, SEQ_PER_CORE * L], F16)
            ekT_sb = wpool.tile([96, G16 * 128], F16)
            tscanP_sb = wpool.tile([96, TLEV * 96], F32)

            def late_weight_dmas():
                nc.sync.dma_start(
                    out=ekT_sb[:].rearrange("j (d k) -> j d k", k=128),
                    in_=ekT_d[:].transpose([1, 0, 2]))
                nc.sync.dma_start(
                    out=phiT_sb[:].rearrange("p (s m) -> p s m", m=L),
                    in_=phiT_d[:].transpose([1, 0, 2]))
                nc.sync.dma_start(
                    out=tscanP_sb[:].rearrange("j (d k) -> j d k", k=96),
                    in_=tscanP_d[:].transpose([1, 0, 2]))
                nc.sync.dma_start(
                    out=g2_sb[:].rearrange("k (s m) -> k s m", m=L),
                    in_=g2_d[:].transpose([1, 0, 2]))

            # bufs=2 on xt/ysb: rep k+1's input DMAs and phase A overlap
            # rep k's phase-B tail (steady-state pipelining across reps)
            with tc.tile_pool(name="xt", bufs=2) as xtpool, \
                 tc.tile_pool(name="ysb", bufs=2) as ypool, \
                 tc.tile_pool(name="state", bufs=1) as stpool:
                for _ in range(rep):
                    _one_pass(nc, tc, xt_d, y_d, phiT_sb, g2_sb, mT_sb,
                              ekT_sb, tscanP_sb, xtpool, ypool, stpool,
                              late_weight_dmas)
                    late_weight_dmas = None
    nc.compile()
    _NC_CACHE[key] = nc
    return nc


def _one_pass(nc, tc, xt_d, y_d, phiT_sb, g2_sb, mT_sb, ekT_sb, tscanP_sb,
              xtpool, ypool, stpool, late_weight_dmas=None):
    XT = [xtpool.tile([ROWS, COLS], F16, tag=f"xt{s}", name=f"xt{s}")
          for s in range(SEQ_PER_CORE)]
    YS = [ypool.tile([ROWS, COLS], F16, tag=f"ys{s}", name=f"ys{s}")
          for s in range(SEQ_PER_CORE)]
    # Uq: q-ordered fp16 U, col j*128+p <-> chunk c = 16p+j;
    # rows 12s..12s+12 = seq s.
    Uq = stpool.tile([96, NCH], F16, tag="uq")
    Sq = stpool.tile([96, NCH], F16, tag="sq")
    Bb = stpool.tile([96, NG], F32, tag="bb")     # group-scan state (f32)
    B16 = stpool.tile([96, NG], F16, tag="b16")   # exclusive, fp16

    for sq in range(SEQ_PER_CORE):
        nc.sync.dma_start(out=XT[sq], in_=xt_d[sq])
    if late_weight_dmas is not None:
        late_weight_dmas()

    # ---- phase A: U = M x, all 8 seqs accumulated into 96-row psum tiles;
    # psum block i holds q-columns [512i, 512i+512) contiguously.
    with tc.tile_pool(name="up", bufs=1, space="PSUM") as upsum:
        ups = [upsum.tile([96, 512], F32, tag=f"up{i}", name=f"up{i}")
               for i in range(4)]
        for sq in range(SEQ_PER_CORE):
            for i in range(4):
                nc.tensor.matmul(
                    ups[i][:],
                    lhsT=mT_sb[:, sq * 96:(sq + 1) * 96],
                    rhs=XT[sq][:, i * 512:(i + 1) * 512],
                    start=(sq == 0), stop=(sq == SEQ_PER_CORE - 1))
        for i in range(4):
            eng = nc.vector if i % 2 == 0 else nc.scalar
            if eng is nc.vector:
                eng.tensor_copy(out=Uq[:, 512 * i:512 * (i + 1)],
                                in_=ups[i][:])
            else:
                eng.copy(Uq[:, 512 * i:512 * (i + 1)], ups[i][:])

    with tc.tile_pool(name="sqp", bufs=1, space="PSUM") as sqpool, \
         tc.tile_pool(name="vt", bufs=1, space="PSUM") as vtpool, \
         tc.tile_pool(name="yp", bufs=3, space="PSUM") as yppool:
        # ---- group sums: V_p = sum_j E^(15-j) U_(16p+j), PSUM-accumulated
        V = vtpool.tile([128, NG], F32, tag="vt", name="vt")
        for jp in range(G16):
            k = 15 - jp
            nc.tensor.matmul(
                V[:],
                lhsT=ekT_sb[:, k * 128:(k + 1) * 128],
                rhs=Uq[:, 128 * jp:128 * (jp + 1)],
                start=(jp == 0), stop=(jp == G16 - 1))
        nc.vector.tensor_copy(out=Bb[:, :], in_=V[0:96, :])

        # ---- assemble U-terms into SQ psum: block j += E^(j-1-j') Uq_(j'),
        # emitted k-major (k = j-1-j' >= 1; the k = 0 term is folded into
        # the final DVE add as a plain Uq block shift).  These only need Uq,
        # so they fill the PE while the group scan below runs.
        sqt = sqpool.tile([128, NCH], F32, tag="sqp", name="sqt")
        # for fixed k, the out blocks j = k+1..15 and rhs blocks j-1-k =
        # 0..14-k are both contiguous col spans: one shifted matmul per k,
        # split only at 512-col psum bank boundaries
        uterms = []
        for k in range(1, 15):
            o0, o1 = 128 * (k + 1), NCH
            c0 = o0
            while c0 < o1:
                c1 = min((c0 // 512 + 1) * 512, o1)
                uterms.append((k, c0, c1))
                c0 = c1
        ut_idx = [0]

        def emit_uterms(n):
            for _ in range(n):
                if ut_idx[0] >= len(uterms):
                    return
                k, c0, c1 = uterms[ut_idx[0]]
                ut_idx[0] += 1
                sh = 128 * (k + 1)
                nc.tensor.matmul(
                    sqt[:, c0:c1],
                    lhsT=ekT_sb[:, k * 128:(k + 1) * 128],
                    rhs=Uq[:, c0 - sh:c1 - sh],
                    start=(k == 1), stop=False)

        # ---- group scan: 7-level KS in f32, transitions ((E^16)^(2^d))^T,
        # interleaved with the U-term matmuls (PE fill during DVE adds)
        for d in range(TLEV):
            sh = 1 << d
            tsp = vtpool.tile([128, NG], F32, tag="vt", name="tsp")
            nc.tensor.matmul(
                tsp[0:96, 0:NG - sh],
                lhsT=tscanP_sb[:, d * 96:(d + 1) * 96],
                rhs=Bb[:, 0:NG - sh],
                start=True, stop=True)
            emit_uterms(3)
            nc.vector.tensor_add(out=Bb[:, sh:NG], in0=Bb[:, sh:NG],
                                 in1=tsp[0:96, 0:NG - sh])
        emit_uterms(len(uterms))  # leftovers
        # exclusive shift + fp16: B16 = [0, Bb[0..NG-2]]
        nc.vector.memset(B16[:, 0:1], 0.0)
        nc.vector.tensor_copy(out=B16[:, 1:NG], in_=Bb[:, 0:NG - 1])

        # ---- T-terms: block j += E^j * B16.  PSUM group discipline: each
        # 2KB bank (4 blocks) has ONE group, started by its k==1 U-term
        # (which marks the whole bank pending-zero, so blocks 0/1 overwrite
        # zeros correctly) and stopped by the bank's last T-term.
        for j in range(G16):
            nc.tensor.matmul(
                sqt[:, 128 * j:128 * (j + 1)],
                lhsT=ekT_sb[:, j * 128:(j + 1) * 128],
                rhs=B16[:, :],
                start=False, stop=(j % 4 == 3))

        # ---- final states: Sq block j = psum block j + Uq block j-1.
        # Split jg-aligned so the first gamma tiles unblock after ~0.5us
        # instead of waiting for the full-width add.
        nc.scalar.copy(Sq[:, 0:128], sqt[0:96, 0:128])
        for jg in range(4):
            c0, c1 = max(128, 512 * jg), 512 * (jg + 1)
            nc.vector.tensor_add(out=Sq[:, c0:c1],
                                 in0=Uq[:, c0 - 128:c1 - 128],
                                 in1=sqt[0:96, c0:c1])

        # ---- phase B: y = Phi x + Gamma S fused per 512-col psum tile,
        # then a single psum -> fp16 copy (ACT/DVE alternating)
        for s in range(SEQ_PER_CORE):
            for jg in range(4):
                gp = yppool.tile([128, 512], F32, tag="yp", name="gp")
                # one psum group per 512-col bank: first phi mm starts (bank
                # marked pending-zero), last gamma mm stops
                for jj in range(4):
                    j = 4 * jg + jj
                    nc.tensor.matmul(
                        gp[:, 128 * jj:128 * (jj + 1)],
                        lhsT=XT[s][:, 128 * j:128 * (j + 1)],
                        rhs=phiT_sb[:, 128 * s:128 * (s + 1)],
                        start=(jj == 0), stop=False)
                for jj in range(4):
                    j = 4 * jg + jj
                    nc.tensor.matmul(
                        gp[:, 128 * jj:128 * (jj + 1)],
                        lhsT=Sq[:, 128 * j:128 * (j + 1)],
                        rhs=g2_sb[:, 128 * s:128 * (s + 1)],
                        start=False, stop=(jj == 3))
                if (4 * s + jg) % 2 == 0:
                    nc.vector.tensor_copy(
                        out=YS[s][:, 512 * jg:512 * (jg + 1)], in_=gp[:])
                else:
                    nc.scalar.copy(YS[s][:, 512 * jg:512 * (jg + 1)], gp[:])
            nc.sync.dma_start(out=y_d[s], in_=YS[s])


# ----------------------------------------------------------------- entry point
class BassRunner:
    """Builds the sharded jitted executable for a compiled Bass module once;
    subsequent calls only device_put inputs and execute."""

    def __init__(self, nc, n_cores=N_CORES):
        import jax
        from jax.experimental.shard_map import shard_map
        from jax.sharding import Mesh, PartitionSpec
        from concourse.bass2jax import (_bass_exec_p, install_neuronx_cc_hook,
                                        partition_id_tensor)
        install_neuronx_cc_hook()
        self.jax = jax
        partition_name = (nc.partition_id_tensor.name
                          if nc.partition_id_tensor else None)
        in_names, out_names, out_avals, zero_outs = [], [], [], []
        for alloc in nc.m.functions[0].allocations:
            if not isinstance(alloc, mybir.MemoryLocationSet):
                continue
            name = alloc.memorylocations[0].name
            if alloc.kind == "ExternalInput":
                if name != partition_name:
                    in_names.append(name)
            elif alloc.kind == "ExternalOutput":
                out_names.append(name)
                shape = tuple(alloc.tensor_shape)
                dtype = mybir.dt.np(alloc.dtype)
                out_avals.append(jax.core.ShapedArray(shape, dtype))
                zero_outs.append(np.zeros(shape, dtype))
        self.in_names, self.out_names = in_names, out_names
        self.out_avals, self.zero_outs = out_avals, zero_outs
        all_in_names = list(in_names) + list(out_names)
        if partition_name is not None:
            all_in_names.append(partition_name)

        def _body(*args):
            operands = list(args)
            if partition_name is not None:
                operands.append(partition_id_tensor())
            return tuple(_bass_exec_p.bind(
                *operands, out_avals=tuple(out_avals),
                in_names=tuple(all_in_names), out_names=tuple(out_names),
                lowering_input_output_aliases=(),
                sim_require_finite=True, sim_require_nnan=True, nc=nc))

        devices = jax.devices()[:n_cores]
        mesh = Mesh(np.asarray(devices), ("core",))
        nin = len(in_names) + len(out_names)
        self.fn = jax.jit(
            shard_map(_body, mesh=mesh,
                      in_specs=(PartitionSpec("core"),) * nin,
                      out_specs=(PartitionSpec("core"),) * len(out_names),
                      check_rep=False),
            keep_unused=True)
        self.n_cores = n_cores

    def concat_args(self, in_maps):
        args = [np.concatenate([np.asarray(in_maps[c][nm])
                                for c in range(self.n_cores)], axis=0)
                for nm in self.in_names]
        args += [np.zeros((self.n_cores * z.shape[0], *z.shape[1:]), z.dtype)
                 for z in self.zero_outs]
        return args

    def __call__(self, in_maps):
        outs = self.fn(*self.concat_args(in_maps))
        self.jax.block_until_ready(outs)
        return outs


_RUNNER_CACHE = {}


def _get_runner(rep=1):
    if rep not in _RUNNER_CACHE:
        _RUNNER_CACHE[rep] = BassRunner(build_nc(rep=rep))
    return _RUNNER_CACHE[rep]


def _prepare_in_maps(x, p, W, b, sample_rate):
    bc, ac = _coeffs_from_inputs(p, W, b, sample_rate)
    A, Bv, Cv, D = _state_space(bc, ac)
    Ab, Bb, Cb = _balance(A, Bv, Cv)
    h, Gam, M, Pd = _derived(Ab, Bb, Cb, D)
    nb = Ab.shape[0]
    E = Pd[:, 0]                       # A^128 (balanced basis)
    Ek = np.zeros((nb, G16, NSTATE, NSTATE))
    Ek[:, 0] = np.eye(NSTATE)
    for k in range(1, G16):
        Ek[:, k] = np.einsum("bij,bjk->bik", E, Ek[:, k - 1])
    phiT, g2, mT, ekT, tscanP = _pack_weights(h, Gam, M, Pd, Ek)
    # chunk-column layout: xt[s][n, j*128+p] = x[s, 2048p + 128j + n]
    # (chunk c = 16p + j at column q = j*128 + p, matching the device views)
    x4 = x.reshape(B * C, ROWS, JG, L).astype(np.float32)
    xt = np.ascontiguousarray(x4.transpose(0, 3, 2, 1)).reshape(
        B * C, L, COLS).astype(np.float16)
    in_maps = []
    for core in range(N_CORES):
        sl = slice(core * SEQ_PER_CORE, (core + 1) * SEQ_PER_CORE)
        in_maps.append({
            "xt": np.ascontiguousarray(xt[sl]),
            "phiT": np.ascontiguousarray(phiT[sl]),
            "g2": np.ascontiguousarray(g2[sl]),
            "mT": np.ascontiguousarray(mT[sl]),
            "ekT": np.ascontiguousarray(ekT[core]),
            "tscanP": np.ascontiguousarray(tscanP[core]),
        })
    return in_maps


def kernel(x, p, W, b, sample_rate):
    runner = _get_runner(rep=1)
    in_maps = _prepare_in_maps(x, p, W, b, sample_rate)
    outs = runner(in_maps)
    y = np.asarray(outs[0]).astype(np.float32).reshape(B * C, T)
    return y.reshape(B, C, T)
